# revision 16
# baseline (speedup 1.0000x reference)
"""CenterNet (CtdetLoss) Trainium2 Bass kernel, v2.

Math: with p = pred_hm, t = log1p(-p) * p^2, m4 = (1-hm)^4,
  F - Z = t*(m4-1)  densely, plus  ln(p)*(1-p)^2  at the K-sparse
  positive pixels (hm == 1.0 -> exactly the object centers).
Per-object rectangle sums without SATs:
  rect_k(channel c_k) = sum_y My[k,y] * sum_x Mx[k,x] * G[c_k,y,x]
The y-contraction runs on the TensorEngine. The class selection is
folded into the lhsT: for channel-group g the lhsT column k is
My[k,:] gated by [cls_k in group g], so the per-class psum
accumulates across ALL groups into one [K, CG*W] bank set; a single
Mx*onehot-masked reduce per image finishes the rect. The class-summed
field for S_ZS accumulates through a second lhsT (ungated) with the
4-fold column trick. hm is DMA-cast to bf16 (SWDGE) so the whole
(1-hm)^4-1 chain runs at DVE 2x rate via (hm-2)*hm / (q+2)*q.
Positive-pixel values and reg-L1 values come from indirect row
gathers (512B rows) with one-hot column-select masks.

Sharding: data-parallel over batch, 2 images per core on 8 cores.
Host preprocessing only touches the small int tensors (masks,
one-hots, gather row indices, per-object weights); every FLOP on
dense map data runs on device. Host combines the 8 cores' per-image
partial sums into the final 4 scalars.
"""

import sys

sys.path.insert(0, "/opt/trn_rl_repo")

import numpy as np
import ml_dtypes

B, C, H, W, K = 16, 80, 128, 128, 128
NCORES = 8
NB = B // NCORES          # images per core
CG = 16                   # channels per group
NG = C // CG              # channel groups per image
HM_W, WH_W, OFF_W = 1.0, 0.1, 1.0

BF16 = ml_dtypes.bfloat16

_module_cache = {}


def build_module():
    if "nc" in _module_cache:
        return _module_cache["nc"]

    import concourse.bacc as bacc
    import concourse.bass as bass
    import concourse.tile as tile
    from concourse import mybir

    f32 = mybir.dt.float32
    bf16 = mybir.dt.bfloat16
    i32 = mybir.dt.int32
    Alu = mybir.AluOpType
    Act = mybir.ActivationFunctionType
    Ax = mybir.AxisListType

    nc = bacc.Bacc(None, target_bir_lowering=False)

    # ---- DRAM I/O ----
    phm = nc.dram_tensor("phm", [NB, C, H, W], f32, kind="ExternalInput")
    hmt = nc.dram_tensor("hm", [NB, C, H, W], f32, kind="ExternalInput")
    pwh = nc.dram_tensor("pwh", [NB, 2, H, W], f32, kind="ExternalInput")
    prg = nc.dram_tensor("prg", [NB, 2, H, W], f32, kind="ExternalInput")
    myt5 = nc.dram_tensor("myt5", [NB, H, NG * K], bf16, kind="ExternalInput")
    mytz = nc.dram_tensor("mytz", [NB, H, K], bf16, kind="ExternalInput")
    m0 = nc.dram_tensor("m0", [NB, K, CG * W], bf16, kind="ExternalInput")
    mxr4 = nc.dram_tensor("mxr4", [NB, K, 4 * W], bf16, kind="ExternalInput")
    sk = nc.dram_tensor("sk", [NB, K, 1], f32, kind="ExternalInput")
    mts = nc.dram_tensor("mts", [NB, K, K], bf16, kind="ExternalInput")
    rpos = nc.dram_tensor("rpos", [NB, K, 1], i32, kind="ExternalInput")
    cxsel = nc.dram_tensor("cxsel", [NB, K, W], f32, kind="ExternalInput")
    rwh = nc.dram_tensor("rwh", [NB, 2, K, 1], i32, kind="ExternalInput")
    rrg = nc.dram_tensor("rrg", [NB, 2, K, 1], i32, kind="ExternalInput")
    csind = nc.dram_tensor("csind", [NB, K, W], f32, kind="ExternalInput")
    m2 = nc.dram_tensor("m2", [NB, K, 2], f32, kind="ExternalInput")
    tmw = nc.dram_tensor("tmw", [NB, K, 2], f32, kind="ExternalInput")
    tmr = nc.dram_tensor("tmr", [NB, K, 2], f32, kind="ExternalInput")
    out = nc.dram_tensor("out", [4, NB], f32, kind="ExternalOutput")

    phm_flat = phm[:].rearrange("b c y x -> (b c y) x")
    pwh_flat = pwh[:].rearrange("b d y x -> (b d y) x")
    prg_flat = prg[:].rearrange("b d y x -> (b d y) x")

    with tile.TileContext(nc) as tc:
        with (
            tc.tile_pool(name="consts", bufs=1) as consts,
            tc.tile_pool(name="ld", bufs=5) as ld,
            tc.tile_pool(name="work", bufs=3) as work,
            tc.tile_pool(name="scr", bufs=2) as scr,
            tc.tile_pool(name="acc", bufs=1) as acc,
            tc.tile_pool(name="ep", bufs=2) as ep,
            tc.tile_pool(name="psg", bufs=1, space="PSUM") as psgp,
            tc.tile_pool(name="psz", bufs=1, space="PSUM") as pszp,
            tc.tile_pool(name="pss", bufs=1, space="PSUM") as pss,
        ):
            ones_s = consts.tile([K, 1], f32, tag="ones")
            nc.vector.memset(ones_s, 1.0)
            O = acc.tile([4, NB], f32, tag="O")

            for b in range(NB):
                # ---- kick off the first dense loads before anything else ----
                p4_t = [None] * NG
                hmb_t = [None] * NG

                def load_group(g, b=b, p4_t=p4_t, hmb_t=hmb_t):
                    cs = g * CG
                    p4_t[g] = ld.tile([H, CG * W], f32, tag="p4", name=f"p4_{b}_{g}")
                    nc.sync.dma_start(
                        out=p4_t[g][:].rearrange("p (c x) -> p c x", c=CG),
                        in_=phm[b, cs : cs + CG].rearrange("c y x -> y c x"),
                    )
                    hmb_t[g] = ld.tile([H, CG * W], bf16, tag="hmb", name=f"hmb_{b}_{g}")
                    nc.gpsimd.dma_start(
                        out=hmb_t[g][:].rearrange("p (c x) -> p c x", c=CG),
                        in_=hmt[b, cs : cs + CG].rearrange("c y x -> y c x"),
                    )

                load_group(0)
                # ---- per-image constants ----
                myt5_s = consts.tile([H, NG * K], bf16, tag=f"myt5{b}")
                nc.scalar.dma_start(out=myt5_s, in_=myt5[b])
                mytz_s = consts.tile([H, K], bf16, tag=f"mytz{b}")
                nc.scalar.dma_start(out=mytz_s, in_=mytz[b])
                m0_s = consts.tile([K, CG * W], bf16, tag=f"m0{b}")
                nc.scalar.dma_start(out=m0_s, in_=m0[b])
                mxr4_s = consts.tile([K, 4 * W], bf16, tag=f"mxr4{b}")
                nc.scalar.dma_start(out=mxr4_s, in_=mxr4[b])
                sk_s = consts.tile([K, 1], f32, tag=f"sk{b}")
                nc.scalar.dma_start(out=sk_s, in_=sk[b])
                mt_s = consts.tile([K, K], bf16, tag=f"mt{b}")
                nc.scalar.dma_start(out=mt_s, in_=mts[b])
                rpos_s = consts.tile([K, 1], i32, tag=f"rpos{b}")
                nc.scalar.dma_start(out=rpos_s, in_=rpos[b])
                cxsel_s = consts.tile([K, W], f32, tag=f"cxsel{b}")
                nc.scalar.dma_start(out=cxsel_s, in_=cxsel[b])
                rwh_s = [
                    consts.tile([K, 1], i32, tag=f"rwh{b}{d}", name=f"rwh_s{b}{d}")
                    for d in range(2)
                ]
                rrg_s = [
                    consts.tile([K, 1], i32, tag=f"rrg{b}{d}", name=f"rrg_s{b}{d}")
                    for d in range(2)
                ]
                for d in range(2):
                    nc.scalar.dma_start(out=rwh_s[d], in_=rwh[b, d])
                    nc.scalar.dma_start(out=rrg_s[d], in_=rrg[b, d])
                csind_s = consts.tile([K, W], f32, tag=f"csind{b}")
                nc.scalar.dma_start(out=csind_s, in_=csind[b])
                m2_s = consts.tile([K, 2], f32, tag=f"m2{b}")
                nc.scalar.dma_start(out=m2_s, in_=m2[b])
                tmw_s = consts.tile([K, 2], f32, tag=f"tmw{b}")
                nc.scalar.dma_start(out=tmw_s, in_=tmw[b])
                tmr_s = consts.tile([K, 2], f32, tag=f"tmr{b}")
                nc.scalar.dma_start(out=tmr_s, in_=tmr[b])

                psg_acc = psgp.tile([K, CG * W], f32, tag="psgacc")
                psz_acc = pszp.tile([K, 4 * W], f32, tag="pszacc")

                # ---- pre-phase: gather-based terms (independent of the
                # dense pipeline; overlaps the dense loads) ----
                rowg = ep.tile([K, W], f32, tag="rowg")
                nc.gpsimd.indirect_dma_start(
                    out=rowg,
                    out_offset=None,
                    in_=phm_flat,
                    in_offset=bass.IndirectOffsetOnAxis(ap=rpos_s[:], axis=0),
                )
                pj = ep.tile([K, 1], f32, tag="pj")
                scw = scr.tile([K, W], f32, tag="scw")
                nc.vector.scalar_tensor_tensor(
                    scw, rowg, 1.0, cxsel_s,
                    op0=Alu.mult, op1=Alu.mult, accum_out=pj,
                )
                lnp = ep.tile([K, 1], f32, tag="lnp")
                nc.scalar.activation(lnp, pj, Act.Ln)
                q2 = ep.tile([K, 1], f32, tag="q2")
                nc.scalar.activation(q2, pj, Act.Square, bias=1.0, scale=-1.0)
                A = ep.tile([K, 1], bf16, tag="A")
                nc.vector.tensor_mul(A, lnp, q2)
                psp = pss.tile([K, 1], f32, tag="psp")
                nc.tensor.matmul(psp, lhsT=mt_s, rhs=A, start=True, stop=True)
                posG = ep.tile([K, 1], f32, tag="posG")
                nc.scalar.copy(posG, psp)
                Q = ep.tile([K, 4], f32, tag=f"Q{b}")
                nc.vector.memset(Q, 0.0)
                for col, flat, rows, tm in (
                    (1, pwh_flat, rwh_s, tmw_s),
                    (2, prg_flat, rrg_s, tmr_s),
                ):
                    PW = ep.tile([K, 2], f32, tag=f"PW{col}")
                    for d in range(2):
                        rg = ep.tile([K, W], f32, tag=f"rg{col}{d}")
                        nc.gpsimd.indirect_dma_start(
                            out=rg,
                            out_offset=None,
                            in_=flat,
                            in_offset=bass.IndirectOffsetOnAxis(
                                ap=rows[d][:], axis=0
                            ),
                        )
                        scw2 = scr.tile([K, W], f32, tag="scw")
                        nc.vector.scalar_tensor_tensor(
                            scw2, rg, 1.0, csind_s,
                            op0=Alu.mult, op1=Alu.mult,
                            accum_out=PW[:, d : d + 1],
                        )
                    u = ep.tile([K, 2], f32, tag=f"u{col}")
                    nc.vector.tensor_mul(u, PW, m2_s)
                    nc.vector.tensor_sub(u, u, tm)
                    nc.vector.tensor_reduce(
                        Q[:, col : col + 1], u, axis=Ax.X, op=Alu.add,
                        apply_absolute_value=True,
                    )

                # ---- dense channel-group loop ----
                for g in range(NG):
                    if g + 1 < NG:
                        load_group(g + 1)
                    p4 = p4_t[g]
                    hmb = hmb_t[g]
                    # l1 = ln(1-p), p2 = p^2   (ACT, bf16 out)
                    l1 = work.tile([H, CG * W], bf16, tag="l1")
                    nc.scalar.activation(l1, p4, Act.Ln, bias=1.0, scale=-1.0)
                    p2 = work.tile([H, CG * W], bf16, tag="p2")
                    nc.scalar.activation(p2, p4, Act.Square)
                    # t = l1 * p2                  (DVE bf16 2x)
                    t = work.tile([H, CG * W], bf16, tag="t")
                    nc.vector.tensor_mul(t, l1, p2)
                    # m2t = (1-hm)^2 (ACT); m4 = m2t^2 (TT 2x);
                    # u = m4 - 1 (tensor_scalar 4x); g4 = u * t (TT 2x)
                    m2t = work.tile([H, CG * W], bf16, tag="m2t")
                    nc.scalar.activation(m2t, hmb, Act.Square, bias=1.0, scale=-1.0)
                    m4 = work.tile([H, CG * W], bf16, tag="m4")
                    nc.vector.tensor_mul(m4, m2t, m2t)
                    u4 = work.tile([H, CG * W], bf16, tag="u4")
                    nc.vector.tensor_scalar_add(u4, m4, -1.0)
                    g4 = work.tile([H, CG * W], bf16, tag="g4")
                    nc.vector.tensor_mul(g4, u4, t)

                    # S_ZS: psz_acc[k, 0:512] += MyT.T @ t (2048 cols fold
                    # onto 512; residues sum out in the final mxr4 reduce)
                    for hh in range(4):
                        nc.tensor.matmul(
                            psz_acc,
                            lhsT=mytz_s,
                            rhs=t[:, hh * 512 : hh * 512 + 512],
                            start=(g == 0 and hh == 0),
                            stop=(g == NG - 1 and hh == 3),
                            skip_group_check=True,
                        )
                    # per-class: psg_acc[k, :] += (My gated by class-group).T @ g4
                    for hh in range(4):
                        sl = slice(hh * 512, hh * 512 + 512)
                        nc.tensor.matmul(
                            psg_acc[:, sl],
                            lhsT=myt5_s[:, g * K : (g + 1) * K],
                            rhs=g4[:, sl],
                            start=(g == 0),
                            stop=(g == NG - 1),
                            skip_group_check=True,
                        )

                # ---- post-phase: the two psum reduces + combine ----
                rectG = ep.tile([K, 1], f32, tag="rectG")
                scg = scr.tile([K, CG * W], bf16, tag="scg")
                nc.vector.scalar_tensor_tensor(
                    scg, psg_acc, 1.0, m0_s,
                    op0=Alu.mult, op1=Alu.mult, accum_out=rectG,
                )
                szs = ep.tile([K, 1], f32, tag="szs")
                scz = scr.tile([K, 4 * W], f32, tag="scz")
                nc.vector.scalar_tensor_tensor(
                    scz, psz_acc, 1.0, mxr4_s,
                    op0=Alu.mult, op1=Alu.mult, accum_out=szs,
                )
                # total = rectG + posG + S_ZS ;  Q[:,0] = total * s
                tot = ep.tile([K, 1], f32, tag="tot")
                nc.vector.tensor_add(tot, rectG, posG)
                nc.vector.tensor_add(tot, tot, szs)
                nc.vector.tensor_mul(Q[:, 0:1], tot, sk_s)
                # partition-reduce the 4 columns: out[4,1] = Q.T @ ones
                psq = pss.tile([4, 1], f32, tag="psq")
                nc.tensor.matmul(psq, lhsT=Q, rhs=ones_s, start=True, stop=True)
                nc.scalar.copy(O[:, b : b + 1], psq)

            nc.sync.dma_start(out=out[:], in_=O)

    nc.compile()
    _module_cache["nc"] = nc
    return nc


def prep_in_maps(inputs):
    """Host-side prep: shard the dense maps per core, derive mask/index
    constants from the small int tensors."""
    pred_hm = np.asarray(inputs["pred_hm"], np.float32)
    pred_wh = np.asarray(inputs["pred_wh"], np.float32)
    pred_reg = np.asarray(inputs["pred_reg"], np.float32)
    hm = np.asarray(inputs["hm"], np.float32)
    wh_t = np.asarray(inputs["wh_t"], np.float32)
    reg_t = np.asarray(inputs["reg_t"], np.float32)
    reg_mask = np.asarray(inputs["reg_mask"], np.float32)
    ind = np.asarray(inputs["ind"]).astype(np.int64)
    cxcy = np.asarray(inputs["cxcy"]).astype(np.int64)
    ori_wh = np.asarray(inputs["ori_wh"]).astype(np.int64)
    cls_idx = np.asarray(inputs["cls_idx"]).astype(np.int64)

    yy = np.arange(H)
    xx = np.arange(W)
    per_img = []
    for b in range(B):
        cls = cls_idx[b]
        cx, cy = cxcy[b, :, 0], cxcy[b, :, 1]
        w = wh_t[b, :, 0].astype(np.int64)
        h = wh_t[b, :, 1].astype(np.int64)
        y0 = np.maximum(1, cy - h // 2 - 1)
        y1 = np.minimum(H - 1, cy + h // 2 + 1)
        y1 = np.maximum(y1, y0)
        x0 = np.maximum(1, cx - w // 2 - 1)
        x1 = np.minimum(W - 1, cx + w // 2 + 1)
        x1 = np.maximum(x1, x0)

        My = ((yy[None, :] >= y0[:, None]) & (yy[None, :] < y1[:, None]))  # [K, H]
        Mx = ((xx[None, :] >= x0[:, None]) & (xx[None, :] < x1[:, None]))  # [K, W]
        # class-group-gated My^T per channel group
        ggate = (cls // CG)[None, :] == np.arange(NG)[:, None]      # [NG, K]
        Myt5 = (My.T[None, :, :] * ggate[:, None, :])               # [NG, H, K]
        Myt5 = Myt5.transpose(1, 0, 2).reshape(H, NG * K).astype(BF16)
        MytZ = My.T.astype(BF16)                                    # [H, K]
        # M0: Mx placed at column block cls % CG
        M0 = np.zeros((K, CG * W), np.float32)
        blk = (cls % CG).astype(np.int64)
        for k in range(K):
            M0[k, blk[k] * W : (blk[k] + 1) * W] = Mx[k]
        Mxr4 = np.tile(Mx.astype(np.float32), (1, 4))

        aspect = w.astype(np.float32) / h.astype(np.float32)
        ori = ori_wh[b, :, 0].astype(np.float32) / ori_wh[b, :, 1].astype(np.float32)
        bad = ~((aspect > 0.5 * ori) & (aspect < 2.0 * ori))
        badw = np.where(bad, 0.5, 1.0).astype(np.float32)
        valid = reg_mask[b] * (w * h > 0).astype(np.float32)

        # unique positive pixels (duplicated centers collapse in hm)
        flat = cls * (H * W) + cy * W + cx
        _, uidx = np.unique(flat, return_index=True)
        nu = len(uidx)
        cls_u, cy_u, cx_u = cls[uidx], cy[uidx], cx[uidx]
        inY = (cy_u[None, :] >= y0[:, None]) & (cy_u[None, :] < y1[:, None])
        inX = (cx_u[None, :] >= x0[:, None]) & (cx_u[None, :] < x1[:, None])
        sameC = cls[:, None] == cls_u[None, :]
        Mkj = (sameC & inY & inX).astype(np.float32)  # [k, j<nu]
        npos = Mkj.sum(1)
        MT = np.zeros((K, K), np.float32)
        MT[:nu, :] = Mkj.T
        rpos_v = np.zeros((K, 1), np.int32)
        rpos_v[:nu, 0] = (b % NB) * C * H + cls_u * H + cy_u
        cxsel_v = np.zeros((K, W), np.float32)
        cx_pad = np.zeros(K, np.int64)
        cx_pad[:nu] = cx_u
        cxsel_v[np.arange(K), cx_pad] = 1.0

        r = np.where(npos > 0, 1.0 / np.maximum(npos, 1.0), 1.0)
        s = (-(r * badw * valid)).astype(np.float32)

        rind = ind[b] // W
        cind = ind[b] % W
        rwh_v = np.zeros((2, K, 1), np.int32)
        rrg_v = np.zeros((2, K, 1), np.int32)
        for d in range(2):
            rwh_v[d, :, 0] = (b % NB) * 2 * H + d * H + rind
            rrg_v[d, :, 0] = (b % NB) * 2 * H + d * H + rind
        csind_v = np.zeros((K, W), np.float32)
        csind_v[np.arange(K), cind] = 1.0

        m = reg_mask[b]
        M2 = np.stack([m, m], 1).astype(np.float32)
        TMW = (wh_t[b] * m[:, None]).astype(np.float32)
        TMR = (reg_t[b] * m[:, None]).astype(np.float32)
        nobj = float(m.sum())
        c1 = (1.0 / max(nobj, 1.0)) if nobj > 0 else 1.0
        invden = 1.0 / (2.0 * nobj + 1e-4)

        per_img.append(
            dict(
                Myt5=Myt5, MytZ=MytZ, M0=M0.astype(BF16),
                Mxr4=Mxr4.astype(BF16), s=s.reshape(K, 1),
                MT=MT.astype(BF16), rpos=rpos_v, cxsel=cxsel_v,
                rwh=rwh_v, rrg=rrg_v, csind=csind_v, M2=M2,
                TMW=TMW, TMR=TMR, c1=c1, invden=invden,
            )
        )

    in_maps = []
    for core in range(NCORES):
        bs = [core * NB + j for j in range(NB)]
        pi = [per_img[b] for b in bs]
        in_maps.append(
            {
                "phm": np.ascontiguousarray(pred_hm[bs]),
                "hm": np.ascontiguousarray(hm[bs]),
                "pwh": np.ascontiguousarray(pred_wh[bs]),
                "prg": np.ascontiguousarray(pred_reg[bs]),
                "myt5": np.stack([p["Myt5"] for p in pi]),
                "mytz": np.stack([p["MytZ"] for p in pi]),
                "m0": np.stack([p["M0"] for p in pi]),
                "mxr4": np.stack([p["Mxr4"] for p in pi]),
                "sk": np.stack([p["s"] for p in pi]),
                "mts": np.stack([p["MT"] for p in pi]),
                "rpos": np.stack([p["rpos"] for p in pi]),
                "cxsel": np.stack([p["cxsel"] for p in pi]),
                "rwh": np.stack([p["rwh"] for p in pi]),
                "rrg": np.stack([p["rrg"] for p in pi]),
                "csind": np.stack([p["csind"] for p in pi]),
                "m2": np.stack([p["M2"] for p in pi]),
                "tmw": np.stack([p["TMW"] for p in pi]),
                "tmr": np.stack([p["TMR"] for p in pi]),
            }
        )
    aux = dict(
        c1=np.array([p["c1"] for p in per_img]),
        invden=np.array([p["invden"] for p in per_img]),
    )
    return in_maps, aux


def combine_outputs(outs, aux):
    """outs: list of 8 per-core 'out' arrays [4, NB]."""
    q = np.concatenate([o.T for o in outs], 0).astype(np.float64)  # [B, 4]
    q_hm, q_wh, q_rg = q[:, 0], q[:, 1], q[:, 2]
    wh_i = q_wh * aux["invden"]
    off_i = q_rg * aux["invden"]
    final_loss = np.mean(HM_W * q_hm + WH_W * wh_i + OFF_W * off_i)
    final_hm = np.mean(q_hm * aux["c1"])
    final_wh = np.mean(wh_i)
    final_off = np.mean(off_i)
    return (
        np.float32(final_loss),
        np.float32(final_hm),
        np.float32(final_wh),
        np.float32(final_off),
    )


def kernel(**inputs):
    from concourse.bass_utils import run_bass_kernel_spmd

    nc = build_module()
    in_maps, aux = prep_in_maps(inputs)
    res = run_bass_kernel_spmd(nc, in_maps, core_ids=list(range(NCORES)))
    outs = [r["out"] for r in res.results]
    return combine_outputs(outs, aux)


# revision 17
# speedup vs baseline: 1.2857x; 1.2857x over previous
"""CenterNet (CtdetLoss) Trainium2 Bass kernel, v2.

Math: with p = pred_hm, t = log1p(-p) * p^2, m4 = (1-hm)^4,
  F - Z = t*(m4-1)  densely, plus  ln(p)*(1-p)^2  at the K-sparse
  positive pixels (hm == 1.0 -> exactly the object centers).
Per-object rectangle sums without SATs:
  rect_k(channel c_k) = sum_y My[k,y] * sum_x Mx[k,x] * G[c_k,y,x]
The y-contraction runs on the TensorEngine. The class selection is
folded into the lhsT: for channel-group g the lhsT column k is
My[k,:] gated by [cls_k in group g], so the per-class psum
accumulates across ALL groups into one [K, CG*W] bank set; a single
Mx*onehot-masked reduce per image finishes the rect. The class-summed
field for S_ZS accumulates through a second lhsT (ungated) with the
4-fold column trick. hm is DMA-cast to bf16 (SWDGE) so the whole
(1-hm)^4-1 chain runs at DVE 2x rate via (hm-2)*hm / (q+2)*q.
Positive-pixel values and reg-L1 values come from indirect row
gathers (512B rows) with one-hot column-select masks.

Sharding: data-parallel over batch, 2 images per core on 8 cores.
Host preprocessing only touches the small int tensors (masks,
one-hots, gather row indices, per-object weights); every FLOP on
dense map data runs on device. Host combines the 8 cores' per-image
partial sums into the final 4 scalars.
"""

import sys

sys.path.insert(0, "/opt/trn_rl_repo")

import numpy as np
import ml_dtypes

B, C, H, W, K = 16, 80, 128, 128, 128
NCORES = 8
NB = B // NCORES          # images per core
CG = 16                   # channels per group
NG = C // CG              # channel groups per image
HM_W, WH_W, OFF_W = 1.0, 0.1, 1.0

BF16 = ml_dtypes.bfloat16

_module_cache = {}


def build_module():
    if "nc" in _module_cache:
        return _module_cache["nc"]

    import concourse.bacc as bacc
    import concourse.bass as bass
    import concourse.tile as tile
    from concourse import mybir

    f32 = mybir.dt.float32
    bf16 = mybir.dt.bfloat16
    i32 = mybir.dt.int32
    Alu = mybir.AluOpType
    Act = mybir.ActivationFunctionType
    Ax = mybir.AxisListType

    nc = bacc.Bacc(None, target_bir_lowering=False)

    # ---- DRAM I/O ----
    phm = nc.dram_tensor("phm", [NB, C, H, W], f32, kind="ExternalInput")
    hmt = nc.dram_tensor("hm", [NB, C, H, W], f32, kind="ExternalInput")
    pwh = nc.dram_tensor("pwh", [NB, 2, H, W], f32, kind="ExternalInput")
    prg = nc.dram_tensor("prg", [NB, 2, H, W], f32, kind="ExternalInput")
    myt5 = nc.dram_tensor("myt5", [NB, H, NG * K], bf16, kind="ExternalInput")
    mytz = nc.dram_tensor("mytz", [NB, H, K], bf16, kind="ExternalInput")
    m0 = nc.dram_tensor("m0", [NB, K, CG * W], bf16, kind="ExternalInput")
    mxr4 = nc.dram_tensor("mxr4", [NB, K, 4 * W], bf16, kind="ExternalInput")
    sk = nc.dram_tensor("sk", [NB, K, 1], f32, kind="ExternalInput")
    mts = nc.dram_tensor("mts", [NB, K, K], bf16, kind="ExternalInput")
    rpos = nc.dram_tensor("rpos", [NB, K, 1], i32, kind="ExternalInput")
    cxsel = nc.dram_tensor("cxsel", [NB, K, W], f32, kind="ExternalInput")
    rwh = nc.dram_tensor("rwh", [NB, 2, K, 1], i32, kind="ExternalInput")
    rrg = nc.dram_tensor("rrg", [NB, 2, K, 1], i32, kind="ExternalInput")
    csind = nc.dram_tensor("csind", [NB, K, W], f32, kind="ExternalInput")
    m2 = nc.dram_tensor("m2", [NB, K, 2], f32, kind="ExternalInput")
    tmw = nc.dram_tensor("tmw", [NB, K, 2], f32, kind="ExternalInput")
    tmr = nc.dram_tensor("tmr", [NB, K, 2], f32, kind="ExternalInput")
    out = nc.dram_tensor("out", [4, NB], f32, kind="ExternalOutput")

    phm_flat = phm[:].rearrange("b c y x -> (b c y) x")
    pwh_flat = pwh[:].rearrange("b d y x -> (b d y) x")
    prg_flat = prg[:].rearrange("b d y x -> (b d y) x")

    with tile.TileContext(nc) as tc:
        with (
            tc.tile_pool(name="consts", bufs=1) as consts,
            tc.tile_pool(name="ld", bufs=5) as ld,
            tc.tile_pool(name="work", bufs=3) as work,
            tc.tile_pool(name="scr", bufs=2) as scr,
            tc.tile_pool(name="acc", bufs=1) as acc,
            tc.tile_pool(name="ep", bufs=2) as ep,
            tc.tile_pool(name="psg", bufs=1, space="PSUM") as psgp,
            tc.tile_pool(name="psz", bufs=1, space="PSUM") as pszp,
            tc.tile_pool(name="pss", bufs=1, space="PSUM") as pss,
        ):
            ones_s = consts.tile([K, 1], f32, tag="ones")
            nc.vector.memset(ones_s, 1.0)
            O = acc.tile([4, NB], f32, tag="O")

            for b in range(NB):
                # ---- kick off the first dense loads before anything else ----
                p4_t = [None] * NG
                hmb_t = [None] * NG

                def load_group(g, b=b, p4_t=p4_t, hmb_t=hmb_t):
                    cs = g * CG
                    p4_t[g] = ld.tile([H, CG * W], f32, tag="p4", name=f"p4_{b}_{g}")
                    nc.sync.dma_start(
                        out=p4_t[g][:].rearrange("p (c x) -> p c x", c=CG),
                        in_=phm[b, cs : cs + CG].rearrange("c y x -> y c x"),
                    )
                    hmb_t[g] = ld.tile([H, CG * W], bf16, tag="hmb", name=f"hmb_{b}_{g}")
                    nc.gpsimd.dma_start(
                        out=hmb_t[g][:].rearrange("p (c x) -> p c x", c=CG),
                        in_=hmt[b, cs : cs + CG].rearrange("c y x -> y c x"),
                    )

                load_group(0)
                # ---- per-image constants ----
                myt5_s = consts.tile([H, NG * K], bf16, tag=f"myt5{b}")
                nc.sync.dma_start(out=myt5_s, in_=myt5[b])
                mytz_s = consts.tile([H, K], bf16, tag=f"mytz{b}")
                nc.sync.dma_start(out=mytz_s, in_=mytz[b])
                m0_s = consts.tile([K, CG * W], bf16, tag=f"m0{b}")
                nc.sync.dma_start(out=m0_s, in_=m0[b])
                mxr4_s = consts.tile([K, 4 * W], bf16, tag=f"mxr4{b}")
                nc.sync.dma_start(out=mxr4_s, in_=mxr4[b])
                sk_s = consts.tile([K, 1], f32, tag=f"sk{b}")
                nc.sync.dma_start(out=sk_s, in_=sk[b])
                mt_s = consts.tile([K, K], bf16, tag=f"mt{b}")
                nc.sync.dma_start(out=mt_s, in_=mts[b])
                rpos_s = consts.tile([K, 1], i32, tag=f"rpos{b}")
                nc.sync.dma_start(out=rpos_s, in_=rpos[b])
                cxsel_s = consts.tile([K, W], f32, tag=f"cxsel{b}")
                nc.sync.dma_start(out=cxsel_s, in_=cxsel[b])
                rwh_s = [
                    consts.tile([K, 1], i32, tag=f"rwh{b}{d}", name=f"rwh_s{b}{d}")
                    for d in range(2)
                ]
                rrg_s = [
                    consts.tile([K, 1], i32, tag=f"rrg{b}{d}", name=f"rrg_s{b}{d}")
                    for d in range(2)
                ]
                for d in range(2):
                    nc.sync.dma_start(out=rwh_s[d], in_=rwh[b, d])
                    nc.sync.dma_start(out=rrg_s[d], in_=rrg[b, d])
                csind_s = consts.tile([K, W], f32, tag=f"csind{b}")
                nc.sync.dma_start(out=csind_s, in_=csind[b])
                m2_s = consts.tile([K, 2], f32, tag=f"m2{b}")
                nc.sync.dma_start(out=m2_s, in_=m2[b])
                tmw_s = consts.tile([K, 2], f32, tag=f"tmw{b}")
                nc.sync.dma_start(out=tmw_s, in_=tmw[b])
                tmr_s = consts.tile([K, 2], f32, tag=f"tmr{b}")
                nc.sync.dma_start(out=tmr_s, in_=tmr[b])

                psg_acc = psgp.tile([K, CG * W], f32, tag="psgacc")
                psz_acc = pszp.tile([K, 4 * W], f32, tag="pszacc")

                # ---- pre-phase: gather-based terms (independent of the
                # dense pipeline; overlaps the dense loads) ----
                rowg = ep.tile([K, W], f32, tag="rowg")
                nc.gpsimd.indirect_dma_start(
                    out=rowg,
                    out_offset=None,
                    in_=phm_flat,
                    in_offset=bass.IndirectOffsetOnAxis(ap=rpos_s[:], axis=0),
                )
                pj = ep.tile([K, 1], f32, tag="pj")
                scw = scr.tile([K, W], f32, tag="scw")
                nc.vector.scalar_tensor_tensor(
                    scw, rowg, 1.0, cxsel_s,
                    op0=Alu.mult, op1=Alu.mult, accum_out=pj,
                )
                lnp = ep.tile([K, 1], f32, tag="lnp")
                nc.scalar.activation(lnp, pj, Act.Ln)
                q2 = ep.tile([K, 1], f32, tag="q2")
                nc.scalar.activation(q2, pj, Act.Square, bias=1.0, scale=-1.0)
                A = ep.tile([K, 1], bf16, tag="A")
                nc.vector.tensor_mul(A, lnp, q2)
                psp = pss.tile([K, 1], f32, tag="psp")
                nc.tensor.matmul(psp, lhsT=mt_s, rhs=A, start=True, stop=True)
                posG = ep.tile([K, 1], f32, tag="posG")
                nc.scalar.copy(posG, psp)
                Q = ep.tile([K, 4], f32, tag=f"Q{b}")
                nc.vector.memset(Q, 0.0)
                for col, flat, rows, tm in (
                    (1, pwh_flat, rwh_s, tmw_s),
                    (2, prg_flat, rrg_s, tmr_s),
                ):
                    PW = ep.tile([K, 2], f32, tag=f"PW{col}")
                    for d in range(2):
                        rg = ep.tile([K, W], f32, tag=f"rg{col}{d}")
                        nc.gpsimd.indirect_dma_start(
                            out=rg,
                            out_offset=None,
                            in_=flat,
                            in_offset=bass.IndirectOffsetOnAxis(
                                ap=rows[d][:], axis=0
                            ),
                        )
                        scw2 = scr.tile([K, W], f32, tag="scw")
                        nc.vector.scalar_tensor_tensor(
                            scw2, rg, 1.0, csind_s,
                            op0=Alu.mult, op1=Alu.mult,
                            accum_out=PW[:, d : d + 1],
                        )
                    u = ep.tile([K, 2], f32, tag=f"u{col}")
                    nc.vector.tensor_mul(u, PW, m2_s)
                    nc.vector.tensor_sub(u, u, tm)
                    nc.vector.tensor_reduce(
                        Q[:, col : col + 1], u, axis=Ax.X, op=Alu.add,
                        apply_absolute_value=True,
                    )

                # ---- dense channel-group loop ----
                for g in range(NG):
                    if g + 1 < NG:
                        load_group(g + 1)
                    p4 = p4_t[g]
                    hmb = hmb_t[g]
                    # l1 = ln(1-p), p2 = p^2   (ACT, bf16 out)
                    l1 = work.tile([H, CG * W], bf16, tag="l1")
                    nc.scalar.activation(l1, p4, Act.Ln, bias=1.0, scale=-1.0)
                    p2 = work.tile([H, CG * W], bf16, tag="p2")
                    nc.scalar.activation(p2, p4, Act.Square)
                    # t = l1 * p2                  (DVE bf16 2x)
                    t = work.tile([H, CG * W], bf16, tag="t")
                    nc.vector.tensor_mul(t, l1, p2)
                    # m2t = (1-hm)^2 (ACT); m4 = m2t^2 (TT 2x);
                    # u = m4 - 1 (tensor_scalar 4x); g4 = u * t (TT 2x)
                    m2t = work.tile([H, CG * W], bf16, tag="m2t")
                    nc.scalar.activation(m2t, hmb, Act.Square, bias=1.0, scale=-1.0)
                    m4 = work.tile([H, CG * W], bf16, tag="m4")
                    nc.vector.tensor_mul(m4, m2t, m2t)
                    u4 = work.tile([H, CG * W], bf16, tag="u4")
                    nc.vector.tensor_scalar_add(u4, m4, -1.0)
                    g4 = work.tile([H, CG * W], bf16, tag="g4")
                    nc.vector.tensor_mul(g4, u4, t)

                    # S_ZS: psz_acc[k, 0:512] += MyT.T @ t (2048 cols fold
                    # onto 512; residues sum out in the final mxr4 reduce)
                    for hh in range(4):
                        nc.tensor.matmul(
                            psz_acc,
                            lhsT=mytz_s,
                            rhs=t[:, hh * 512 : hh * 512 + 512],
                            start=(g == 0 and hh == 0),
                            stop=(g == NG - 1 and hh == 3),
                            skip_group_check=True,
                        )
                    # per-class: psg_acc[k, :] += (My gated by class-group).T @ g4
                    for hh in range(4):
                        sl = slice(hh * 512, hh * 512 + 512)
                        nc.tensor.matmul(
                            psg_acc[:, sl],
                            lhsT=myt5_s[:, g * K : (g + 1) * K],
                            rhs=g4[:, sl],
                            start=(g == 0),
                            stop=(g == NG - 1),
                            skip_group_check=True,
                        )

                # ---- post-phase: the two psum reduces + combine ----
                rectG = ep.tile([K, 1], f32, tag="rectG")
                scg = scr.tile([K, CG * W], bf16, tag="scg")
                nc.vector.scalar_tensor_tensor(
                    scg, psg_acc, 1.0, m0_s,
                    op0=Alu.mult, op1=Alu.mult, accum_out=rectG,
                )
                szs = ep.tile([K, 1], f32, tag="szs")
                scz = scr.tile([K, 4 * W], f32, tag="scz")
                nc.vector.scalar_tensor_tensor(
                    scz, psz_acc, 1.0, mxr4_s,
                    op0=Alu.mult, op1=Alu.mult, accum_out=szs,
                )
                # total = rectG + posG + S_ZS ;  Q[:,0] = total * s
                tot = ep.tile([K, 1], f32, tag="tot")
                nc.vector.tensor_add(tot, rectG, posG)
                nc.vector.tensor_add(tot, tot, szs)
                nc.vector.tensor_mul(Q[:, 0:1], tot, sk_s)
                # partition-reduce the 4 columns: out[4,1] = Q.T @ ones
                psq = pss.tile([4, 1], f32, tag="psq")
                nc.tensor.matmul(psq, lhsT=Q, rhs=ones_s, start=True, stop=True)
                nc.scalar.copy(O[:, b : b + 1], psq)

            nc.sync.dma_start(out=out[:], in_=O)

    nc.compile()
    _module_cache["nc"] = nc
    return nc


def prep_in_maps(inputs):
    """Host-side prep: shard the dense maps per core, derive mask/index
    constants from the small int tensors."""
    pred_hm = np.asarray(inputs["pred_hm"], np.float32)
    pred_wh = np.asarray(inputs["pred_wh"], np.float32)
    pred_reg = np.asarray(inputs["pred_reg"], np.float32)
    hm = np.asarray(inputs["hm"], np.float32)
    wh_t = np.asarray(inputs["wh_t"], np.float32)
    reg_t = np.asarray(inputs["reg_t"], np.float32)
    reg_mask = np.asarray(inputs["reg_mask"], np.float32)
    ind = np.asarray(inputs["ind"]).astype(np.int64)
    cxcy = np.asarray(inputs["cxcy"]).astype(np.int64)
    ori_wh = np.asarray(inputs["ori_wh"]).astype(np.int64)
    cls_idx = np.asarray(inputs["cls_idx"]).astype(np.int64)

    yy = np.arange(H)
    xx = np.arange(W)
    per_img = []
    for b in range(B):
        cls = cls_idx[b]
        cx, cy = cxcy[b, :, 0], cxcy[b, :, 1]
        w = wh_t[b, :, 0].astype(np.int64)
        h = wh_t[b, :, 1].astype(np.int64)
        y0 = np.maximum(1, cy - h // 2 - 1)
        y1 = np.minimum(H - 1, cy + h // 2 + 1)
        y1 = np.maximum(y1, y0)
        x0 = np.maximum(1, cx - w // 2 - 1)
        x1 = np.minimum(W - 1, cx + w // 2 + 1)
        x1 = np.maximum(x1, x0)

        My = ((yy[None, :] >= y0[:, None]) & (yy[None, :] < y1[:, None]))  # [K, H]
        Mx = ((xx[None, :] >= x0[:, None]) & (xx[None, :] < x1[:, None]))  # [K, W]
        # class-group-gated My^T per channel group
        ggate = (cls // CG)[None, :] == np.arange(NG)[:, None]      # [NG, K]
        Myt5 = (My.T[None, :, :] * ggate[:, None, :])               # [NG, H, K]
        Myt5 = Myt5.transpose(1, 0, 2).reshape(H, NG * K).astype(BF16)
        MytZ = My.T.astype(BF16)                                    # [H, K]
        # M0: Mx placed at column block cls % CG
        M0 = np.zeros((K, CG * W), np.float32)
        blk = (cls % CG).astype(np.int64)
        for k in range(K):
            M0[k, blk[k] * W : (blk[k] + 1) * W] = Mx[k]
        Mxr4 = np.tile(Mx.astype(np.float32), (1, 4))

        aspect = w.astype(np.float32) / h.astype(np.float32)
        ori = ori_wh[b, :, 0].astype(np.float32) / ori_wh[b, :, 1].astype(np.float32)
        bad = ~((aspect > 0.5 * ori) & (aspect < 2.0 * ori))
        badw = np.where(bad, 0.5, 1.0).astype(np.float32)
        valid = reg_mask[b] * (w * h > 0).astype(np.float32)

        # unique positive pixels (duplicated centers collapse in hm)
        flat = cls * (H * W) + cy * W + cx
        _, uidx = np.unique(flat, return_index=True)
        nu = len(uidx)
        cls_u, cy_u, cx_u = cls[uidx], cy[uidx], cx[uidx]
        inY = (cy_u[None, :] >= y0[:, None]) & (cy_u[None, :] < y1[:, None])
        inX = (cx_u[None, :] >= x0[:, None]) & (cx_u[None, :] < x1[:, None])
        sameC = cls[:, None] == cls_u[None, :]
        Mkj = (sameC & inY & inX).astype(np.float32)  # [k, j<nu]
        npos = Mkj.sum(1)
        MT = np.zeros((K, K), np.float32)
        MT[:nu, :] = Mkj.T
        rpos_v = np.zeros((K, 1), np.int32)
        rpos_v[:nu, 0] = (b % NB) * C * H + cls_u * H + cy_u
        cxsel_v = np.zeros((K, W), np.float32)
        cx_pad = np.zeros(K, np.int64)
        cx_pad[:nu] = cx_u
        cxsel_v[np.arange(K), cx_pad] = 1.0

        r = np.where(npos > 0, 1.0 / np.maximum(npos, 1.0), 1.0)
        s = (-(r * badw * valid)).astype(np.float32)

        rind = ind[b] // W
        cind = ind[b] % W
        rwh_v = np.zeros((2, K, 1), np.int32)
        rrg_v = np.zeros((2, K, 1), np.int32)
        for d in range(2):
            rwh_v[d, :, 0] = (b % NB) * 2 * H + d * H + rind
            rrg_v[d, :, 0] = (b % NB) * 2 * H + d * H + rind
        csind_v = np.zeros((K, W), np.float32)
        csind_v[np.arange(K), cind] = 1.0

        m = reg_mask[b]
        M2 = np.stack([m, m], 1).astype(np.float32)
        TMW = (wh_t[b] * m[:, None]).astype(np.float32)
        TMR = (reg_t[b] * m[:, None]).astype(np.float32)
        nobj = float(m.sum())
        c1 = (1.0 / max(nobj, 1.0)) if nobj > 0 else 1.0
        invden = 1.0 / (2.0 * nobj + 1e-4)

        per_img.append(
            dict(
                Myt5=Myt5, MytZ=MytZ, M0=M0.astype(BF16),
                Mxr4=Mxr4.astype(BF16), s=s.reshape(K, 1),
                MT=MT.astype(BF16), rpos=rpos_v, cxsel=cxsel_v,
                rwh=rwh_v, rrg=rrg_v, csind=csind_v, M2=M2,
                TMW=TMW, TMR=TMR, c1=c1, invden=invden,
            )
        )

    in_maps = []
    for core in range(NCORES):
        bs = [core * NB + j for j in range(NB)]
        pi = [per_img[b] for b in bs]
        in_maps.append(
            {
                "phm": np.ascontiguousarray(pred_hm[bs]),
                "hm": np.ascontiguousarray(hm[bs]),
                "pwh": np.ascontiguousarray(pred_wh[bs]),
                "prg": np.ascontiguousarray(pred_reg[bs]),
                "myt5": np.stack([p["Myt5"] for p in pi]),
                "mytz": np.stack([p["MytZ"] for p in pi]),
                "m0": np.stack([p["M0"] for p in pi]),
                "mxr4": np.stack([p["Mxr4"] for p in pi]),
                "sk": np.stack([p["s"] for p in pi]),
                "mts": np.stack([p["MT"] for p in pi]),
                "rpos": np.stack([p["rpos"] for p in pi]),
                "cxsel": np.stack([p["cxsel"] for p in pi]),
                "rwh": np.stack([p["rwh"] for p in pi]),
                "rrg": np.stack([p["rrg"] for p in pi]),
                "csind": np.stack([p["csind"] for p in pi]),
                "m2": np.stack([p["M2"] for p in pi]),
                "tmw": np.stack([p["TMW"] for p in pi]),
                "tmr": np.stack([p["TMR"] for p in pi]),
            }
        )
    aux = dict(
        c1=np.array([p["c1"] for p in per_img]),
        invden=np.array([p["invden"] for p in per_img]),
    )
    return in_maps, aux


def combine_outputs(outs, aux):
    """outs: list of 8 per-core 'out' arrays [4, NB]."""
    q = np.concatenate([o.T for o in outs], 0).astype(np.float64)  # [B, 4]
    q_hm, q_wh, q_rg = q[:, 0], q[:, 1], q[:, 2]
    wh_i = q_wh * aux["invden"]
    off_i = q_rg * aux["invden"]
    final_loss = np.mean(HM_W * q_hm + WH_W * wh_i + OFF_W * off_i)
    final_hm = np.mean(q_hm * aux["c1"])
    final_wh = np.mean(wh_i)
    final_off = np.mean(off_i)
    return (
        np.float32(final_loss),
        np.float32(final_hm),
        np.float32(final_wh),
        np.float32(final_off),
    )


def kernel(**inputs):
    from concourse.bass_utils import run_bass_kernel_spmd

    nc = build_module()
    in_maps, aux = prep_in_maps(inputs)
    res = run_bass_kernel_spmd(nc, in_maps, core_ids=list(range(NCORES)))
    outs = [r["out"] for r in res.results]
    return combine_outputs(outs, aux)
